# revision 2
# baseline (speedup 1.0000x reference)
"""Trainium2 Bass kernel: scatter-add of table rows into a voxel grid.

Computes out[cell] += table[row] for ~1M (cell, row) events, out shape
[B*W*H*L, D] = [131072, 256] fp32.

Strategy (8 NeuronCores, output-sharded):
  - Host: sort events by destination cell; core k owns cells
    [k*16384, (k+1)*16384).  Within a core, cells are grouped into 128
    tiles of 128 cells; each tile's events are padded to K*128 (K =
    global max chunks per tile) so all cores run an identical program.
  - Device (per core): dma_gather the event rows (bf16 table, 1024
    rows per call — SWDGE ring limit), build one-hot [event x cell]
    selection matrices on the vector engine, and accumulate
    one-hot^T @ rows on the PE array into a PSUM tile (fp32
    accumulation), then DMA each [128, 256] cell tile to HBM.
"""

import numpy as np
import ml_dtypes

B, W, H, L, D = 4, 32, 32, 32, 256
NCELLS = B * W * H * L          # 131072
TROWS = 4096
NCORES = 8
CPC = NCELLS // NCORES          # cells per core: 16384
TPC = CPC // 128                # 128-cell tiles per core: 128
GIDX = 1024                     # idxs per dma_gather call (HW ring limit)
GCH = GIDX // 128               # event-chunks per gather call: 8

_compiled = {}


def _build(K):
    import concourse.tile as tile
    from concourse import bacc, mybir

    f32, bf16, i16 = mybir.dt.float32, mybir.dt.bfloat16, mybir.dt.int16
    nch = TPC * K                        # event chunks per core
    ncalls = nch // GCH                  # gather calls per core

    nc = bacc.Bacc("TRN2", target_bir_lowering=False, debug=False,
                   num_devices=NCORES)
    tabbf = nc.dram_tensor("tabbf", [TROWS, D], bf16, kind="ExternalInput")
    rows_w = nc.dram_tensor("rows_w", [128, ncalls * (GIDX // 16)], i16,
                            kind="ExternalInput")
    lrel = nc.dram_tensor("lrel", [128, nch], bf16, kind="ExternalInput")
    out = nc.dram_tensor("out", [CPC, D], f32, kind="ExternalOutput")

    with tile.TileContext(nc) as tc:
        with tc.tile_pool(name="const", bufs=1) as constp, \
             tc.tile_pool(name="gbuf", bufs=6) as gpool, \
             tc.tile_pool(name="oh", bufs=6) as ohpool, \
             tc.tile_pool(name="psum", bufs=4, space="PSUM") as pspool, \
             tc.tile_pool(name="stage", bufs=3) as stpool:
            rows_sb = constp.tile([128, ncalls * (GIDX // 16)], i16)
            nc.sync.dma_start(rows_sb[:], rows_w[:])
            lrel_sb = constp.tile([128, nch], bf16)
            nc.sync.dma_start(lrel_sb[:], lrel[:])
            iota_t = constp.tile([128, 128], bf16)
            nc.gpsimd.iota(iota_t[:], pattern=[[1, 128]], base=0,
                           channel_multiplier=0,
                           allow_small_or_imprecise_dtypes=True)

            gt = None
            for t in range(TPC):
                ps = pspool.tile([128, D], f32, space="PSUM")
                for j in range(K):
                    c = t * K + j
                    if c % GCH == 0:
                        ci = c // GCH
                        gt = gpool.tile([128, GCH, D], bf16)
                        nc.gpsimd.dma_gather(
                            gt[:], tabbf[:],
                            rows_sb[:, ci * (GIDX // 16):(ci + 1) * (GIDX // 16)],
                            GIDX, GIDX, D)
                    oh = ohpool.tile([128, 128], bf16)
                    nc.vector.tensor_tensor(
                        out=oh[:],
                        in0=lrel_sb[:, c:c + 1].to_broadcast([128, 128]),
                        in1=iota_t[:],
                        op=mybir.AluOpType.is_equal)
                    nc.tensor.matmul(out=ps[:], lhsT=oh[:],
                                     rhs=gt[:, c % GCH, :],
                                     start=(j == 0), stop=(j == K - 1))
                st = stpool.tile([128, D], f32)
                nc.any.tensor_copy(st[:], ps[:])
                nc.sync.dma_start(out[t * 128:(t + 1) * 128, :], st[:])
    nc.compile()
    return nc


def _marshal(event_cell, event_row):
    """Sort events by cell, bucket into per-(core,tile) padded streams."""
    ecell = np.asarray(event_cell).astype(np.int64)
    erow = np.asarray(event_row).astype(np.int64)
    order = np.argsort(ecell, kind="stable")
    scell = ecell[order]
    srow = erow[order].astype(np.int16)

    ntiles = NCELLS // 128
    bounds = np.searchsorted(scell, np.arange(ntiles + 1) * 128)
    counts = np.diff(bounds)
    K = max(1, int(-(-int(counts.max()) // 128)))
    # chunks per core must divide evenly into gather calls of GCH chunks
    while (TPC * K) % GCH:
        K += 1
    cap = 128 * K

    rows_p = np.zeros((ntiles, cap), np.int16)
    lrel_p = np.full((ntiles, cap), -1.0, np.float32)
    for t in range(ntiles):
        n = int(counts[t])
        if n:
            s = int(bounds[t])
            rows_p[t, :n] = srow[s:s + n]
            lrel_p[t, :n] = scell[s:s + n] & 127

    in_maps = []
    for c in range(NCORES):
        tok = rows_p[c * TPC:(c + 1) * TPC].reshape(-1)      # TPC*cap events
        # wrapped-by-16 idx layout per 1024-idx gather call, replicated
        # to 128 partitions (8 Q7 cores x 16)
        wr = tok.reshape(-1, GIDX).reshape(-1, 64, 16)       # [ncalls, 64, 16]
        wr = wr.transpose(0, 2, 1).reshape(-1, 16, 64)       # call-major [16 x 64]
        wr = np.concatenate(list(wr), axis=1)                # [16, ncalls*64]
        wr = np.tile(wr, (8, 1))                             # [128, ncalls*64]
        lc = lrel_p[c * TPC:(c + 1) * TPC].reshape(TPC * K, 128).T
        in_maps.append({
            "rows_w": np.ascontiguousarray(wr),
            "lrel": np.ascontiguousarray(lc.astype(ml_dtypes.bfloat16)),
        })
    return in_maps, K


def kernel(table, event_cell, event_row, _want_trace=False):
    from concourse.bass_utils import run_bass_kernel_spmd

    tabbf = np.ascontiguousarray(
        np.asarray(table, dtype=np.float32).astype(ml_dtypes.bfloat16))
    in_maps, K = _marshal(event_cell, event_row)
    for m in in_maps:
        m["tabbf"] = tabbf

    if K not in _compiled:
        _compiled[K] = _build(K)
    nc = _compiled[K]

    kw = {"trace": True} if _want_trace else {}
    res = run_bass_kernel_spmd(nc, in_maps, core_ids=list(range(NCORES)), **kw)
    full = np.concatenate([res.results[i]["out"] for i in range(NCORES)], axis=0)
    out = full.reshape(B, W, H, L, D).astype(np.float32)
    if _want_trace:
        return out, res
    return out


# revision 4
# speedup vs baseline: 2.3601x; 2.3601x over previous
"""Trainium2 Bass kernel: scatter-add of table rows into a voxel grid.

Computes out[cell] += table[row] for ~1M (cell, row) events, out shape
[B*W*H*L, D] = [131072, 256] fp32.

Strategy (8 NeuronCores, output-sharded):
  - Host: sort events by destination cell; core k owns cells
    [k*16384, (k+1)*16384).  Within a core, cells are grouped into 128
    tiles of 128 cells; each tile's events are padded to K*128 (K =
    global max chunks per tile) so all cores run an identical program.
  - Device (per core): dma_gather the event rows (bf16 table, 1024
    rows per call — SWDGE ring limit), build one-hot [event x cell]
    selection matrices on the vector engine, and accumulate
    one-hot^T @ rows on the PE array into a PSUM tile (fp32
    accumulation), then DMA each [128, 256] cell tile to HBM.
"""

import numpy as np
import ml_dtypes

B, W, H, L, D = 4, 32, 32, 32, 256
NCELLS = B * W * H * L          # 131072
TROWS = 4096
NCORES = 8
CPC = NCELLS // NCORES          # cells per core: 16384
TPC = CPC // 128                # 128-cell tiles per core: 128
GIDX = 1024                     # idxs per dma_gather call (HW ring limit)
GCH = GIDX // 128               # event-chunks per gather call: 8

_compiled = {}


def _build(K):
    import concourse.tile as tile
    from concourse import bacc, mybir

    f32, bf16, i16 = mybir.dt.float32, mybir.dt.bfloat16, mybir.dt.int16
    nch = TPC * K                        # event chunks per core
    ncalls = nch // GCH                  # gather calls per core

    nc = bacc.Bacc("TRN2", target_bir_lowering=False, debug=False,
                   num_devices=NCORES, num_swdge_queues=4)
    tabbf = nc.dram_tensor("tabbf", [TROWS, D], bf16, kind="ExternalInput")
    rows_w = nc.dram_tensor("rows_w", [128, ncalls * (GIDX // 16)], i16,
                            kind="ExternalInput")
    lrel = nc.dram_tensor("lrel", [128, nch], bf16, kind="ExternalInput")
    out = nc.dram_tensor("out", [CPC, D], f32, kind="ExternalOutput")

    with tile.TileContext(nc) as tc:
        with tc.tile_pool(name="const", bufs=1) as constp, \
             tc.tile_pool(name="gbuf", bufs=6) as gpool, \
             tc.tile_pool(name="oh", bufs=6) as ohpool, \
             tc.tile_pool(name="psum", bufs=4, space="PSUM") as pspool, \
             tc.tile_pool(name="stage", bufs=3) as stpool:
            rows_sb = constp.tile([128, ncalls * (GIDX // 16)], i16)
            nc.sync.dma_start(rows_sb[:], rows_w[:])
            lrel_sb = constp.tile([128, nch], bf16)
            nc.sync.dma_start(lrel_sb[:], lrel[:])
            # iota repeated OHB times along free dim for batched one-hot builds
            OHB = 4
            iota_t = constp.tile([128, OHB, 128], bf16)
            nc.gpsimd.iota(iota_t[:], pattern=[[0, OHB], [1, 128]], base=0,
                           channel_multiplier=0,
                           allow_small_or_imprecise_dtypes=True)

            gt = None
            oh = None
            for t in range(TPC):
                ps = pspool.tile([128, D], f32, space="PSUM")
                for j in range(K):
                    c = t * K + j
                    if c % GCH == 0:
                        ci = c // GCH
                        gt = gpool.tile([128, GCH, D], bf16)
                        nc.gpsimd.dma_gather(
                            gt[:], tabbf[:],
                            rows_sb[:, ci * (GIDX // 16):(ci + 1) * (GIDX // 16)],
                            GIDX, GIDX, D, queue_num=ci % 4)
                    if c % OHB == 0:
                        oh = ohpool.tile([128, OHB, 128], bf16)
                        nb = min(OHB, TPC * K - c)
                        nc.vector.tensor_tensor(
                            out=oh[:, :nb, :],
                            in0=lrel_sb[:, c:c + nb, None].to_broadcast(
                                [128, nb, 128]),
                            in1=iota_t[:, :nb, :],
                            op=mybir.AluOpType.is_equal)
                    nc.tensor.matmul(out=ps[:], lhsT=oh[:, c % OHB, :],
                                     rhs=gt[:, c % GCH, :],
                                     start=(j == 0), stop=(j == K - 1))
                st = stpool.tile([128, D], f32)
                nc.any.tensor_copy(st[:], ps[:])
                nc.sync.dma_start(out[t * 128:(t + 1) * 128, :], st[:])
    nc.compile()
    return nc


def _marshal(event_cell, event_row):
    """Sort events by cell, bucket into per-(core,tile) padded streams."""
    ecell = np.asarray(event_cell).astype(np.int64)
    erow = np.asarray(event_row).astype(np.int64)
    order = np.argsort(ecell, kind="stable")
    scell = ecell[order]
    srow = erow[order].astype(np.int16)

    ntiles = NCELLS // 128
    bounds = np.searchsorted(scell, np.arange(ntiles + 1) * 128)
    counts = np.diff(bounds)
    K = max(1, int(-(-int(counts.max()) // 128)))
    # chunks per core must divide evenly into gather calls of GCH chunks
    while (TPC * K) % GCH:
        K += 1
    cap = 128 * K

    rows_p = np.zeros((ntiles, cap), np.int16)
    lrel_p = np.full((ntiles, cap), -1.0, np.float32)
    for t in range(ntiles):
        n = int(counts[t])
        if n:
            s = int(bounds[t])
            rows_p[t, :n] = srow[s:s + n]
            lrel_p[t, :n] = scell[s:s + n] & 127

    in_maps = []
    for c in range(NCORES):
        tok = rows_p[c * TPC:(c + 1) * TPC].reshape(-1)      # TPC*cap events
        # wrapped-by-16 idx layout per 1024-idx gather call, replicated
        # to 128 partitions (8 Q7 cores x 16)
        wr = tok.reshape(-1, GIDX).reshape(-1, 64, 16)       # [ncalls, 64, 16]
        wr = wr.transpose(0, 2, 1).reshape(-1, 16, 64)       # call-major [16 x 64]
        wr = np.concatenate(list(wr), axis=1)                # [16, ncalls*64]
        wr = np.tile(wr, (8, 1))                             # [128, ncalls*64]
        lc = lrel_p[c * TPC:(c + 1) * TPC].reshape(TPC * K, 128).T
        in_maps.append({
            "rows_w": np.ascontiguousarray(wr),
            "lrel": np.ascontiguousarray(lc.astype(ml_dtypes.bfloat16)),
        })
    return in_maps, K


def kernel(table, event_cell, event_row, _want_trace=False):
    from concourse.bass_utils import run_bass_kernel_spmd

    tabbf = np.ascontiguousarray(
        np.asarray(table, dtype=np.float32).astype(ml_dtypes.bfloat16))
    in_maps, K = _marshal(event_cell, event_row)
    for m in in_maps:
        m["tabbf"] = tabbf

    if K not in _compiled:
        _compiled[K] = _build(K)
    nc = _compiled[K]

    kw = {"trace": True} if _want_trace else {}
    res = run_bass_kernel_spmd(nc, in_maps, core_ids=list(range(NCORES)), **kw)
    full = np.concatenate([res.results[i]["out"] for i in range(NCORES)], axis=0)
    out = full.reshape(B, W, H, L, D).astype(np.float32)
    if _want_trace:
        return out, res
    return out


# revision 5
# speedup vs baseline: 2.7719x; 1.1745x over previous
"""Trainium2 Bass kernel: scatter-add of table rows into a voxel grid.

Computes out[cell] += table[row] for ~1M (cell, row) events, out shape
[B*W*H*L, D] = [131072, 256] fp32.

Strategy (8 NeuronCores, output-sharded):
  - Host: sort events by destination cell; core k owns cells
    [k*16384, (k+1)*16384).  Within a core, cells are grouped into 128
    tiles of 128 cells; each tile's events are padded to K*128 (K =
    global max chunks per tile) so all cores run an identical program.
  - Device (per core): dma_gather the event rows (bf16 table, 1024
    rows per call — SWDGE ring limit), build one-hot [event x cell]
    selection matrices on the vector engine, and accumulate
    one-hot^T @ rows on the PE array into a PSUM tile (fp32
    accumulation), then DMA each [128, 256] cell tile to HBM.
"""

import numpy as np
import ml_dtypes

B, W, H, L, D = 4, 32, 32, 32, 256
NCELLS = B * W * H * L          # 131072
TROWS = 4096
NCORES = 8
CPC = NCELLS // NCORES          # cells per core: 16384
TPC = CPC // 128                # 128-cell tiles per core: 128
GIDX = 1024                     # idxs per dma_gather call (HW ring limit)
GCH = GIDX // 128               # event-chunks per gather call: 8

_compiled = {}


def _build(K):
    import concourse.tile as tile
    from concourse import bacc, mybir

    f32, bf16, i16 = mybir.dt.float32, mybir.dt.bfloat16, mybir.dt.int16
    nch = TPC * K                        # event chunks per core
    ncalls = nch // GCH                  # gather calls per core

    nc = bacc.Bacc("TRN2", target_bir_lowering=False, debug=False,
                   num_devices=NCORES, num_swdge_queues=4)
    tabbf = nc.dram_tensor("tabbf", [TROWS, D], bf16, kind="ExternalInput")
    rows_w = nc.dram_tensor("rows_w", [128, ncalls * (GIDX // 16)], i16,
                            kind="ExternalInput")
    lrel = nc.dram_tensor("lrel", [128, nch], bf16, kind="ExternalInput")
    out = nc.dram_tensor("out", [CPC, D], f32, kind="ExternalOutput")

    with tile.TileContext(nc) as tc:
        with tc.tile_pool(name="const", bufs=1) as constp, \
             tc.tile_pool(name="gbuf", bufs=12) as gpool, \
             tc.tile_pool(name="oh", bufs=8) as ohpool, \
             tc.tile_pool(name="psum", bufs=6, space="PSUM") as pspool, \
             tc.tile_pool(name="stage", bufs=3) as stpool:
            rows_sb = constp.tile([128, ncalls * (GIDX // 16)], i16)
            nc.sync.dma_start(rows_sb[:], rows_w[:])
            lrel_sb = constp.tile([128, nch], bf16)
            nc.sync.dma_start(lrel_sb[:], lrel[:])
            # iota repeated OHB times along free dim for batched one-hot builds
            OHB = 4
            iota_t = constp.tile([128, OHB, 128], bf16)
            nc.gpsimd.iota(iota_t[:], pattern=[[0, OHB], [1, 128]], base=0,
                           channel_multiplier=0,
                           allow_small_or_imprecise_dtypes=True)

            gt = None
            oh = None
            for t in range(TPC):
                ps = pspool.tile([128, D], f32, space="PSUM")
                for j in range(K):
                    c = t * K + j
                    if c % GCH == 0:
                        ci = c // GCH
                        gt = gpool.tile([128, GCH, D], bf16)
                        nc.gpsimd.dma_gather(
                            gt[:], tabbf[:],
                            rows_sb[:, ci * (GIDX // 16):(ci + 1) * (GIDX // 16)],
                            GIDX, GIDX, D, queue_num=ci % 4)
                    if c % OHB == 0:
                        oh = ohpool.tile([128, OHB, 128], bf16)
                        nb = min(OHB, TPC * K - c)
                        nc.vector.tensor_tensor(
                            out=oh[:, :nb, :],
                            in0=lrel_sb[:, c:c + nb, None].to_broadcast(
                                [128, nb, 128]),
                            in1=iota_t[:, :nb, :],
                            op=mybir.AluOpType.is_equal)
                    nc.tensor.matmul(out=ps[:], lhsT=oh[:, c % OHB, :],
                                     rhs=gt[:, c % GCH, :],
                                     start=(j == 0), stop=(j == K - 1))
                st = stpool.tile([128, D], f32)
                nc.any.tensor_copy(st[:], ps[:])
                nc.sync.dma_start(out[t * 128:(t + 1) * 128, :], st[:])
    nc.compile()
    return nc


def _marshal(event_cell, event_row):
    """Sort events by cell, bucket into per-(core,tile) padded streams."""
    ecell = np.asarray(event_cell).astype(np.int64)
    erow = np.asarray(event_row).astype(np.int64)
    order = np.argsort(ecell, kind="stable")
    scell = ecell[order]
    srow = erow[order].astype(np.int16)

    ntiles = NCELLS // 128
    bounds = np.searchsorted(scell, np.arange(ntiles + 1) * 128)
    counts = np.diff(bounds)
    K = max(1, int(-(-int(counts.max()) // 128)))
    # chunks per core must divide evenly into gather calls of GCH chunks
    while (TPC * K) % GCH:
        K += 1
    cap = 128 * K

    rows_p = np.zeros((ntiles, cap), np.int16)
    lrel_p = np.full((ntiles, cap), -1.0, np.float32)
    for t in range(ntiles):
        n = int(counts[t])
        if n:
            s = int(bounds[t])
            rows_p[t, :n] = srow[s:s + n]
            lrel_p[t, :n] = scell[s:s + n] & 127

    in_maps = []
    for c in range(NCORES):
        tok = rows_p[c * TPC:(c + 1) * TPC].reshape(-1)      # TPC*cap events
        # wrapped-by-16 idx layout per 1024-idx gather call, replicated
        # to 128 partitions (8 Q7 cores x 16)
        wr = tok.reshape(-1, GIDX).reshape(-1, 64, 16)       # [ncalls, 64, 16]
        wr = wr.transpose(0, 2, 1).reshape(-1, 16, 64)       # call-major [16 x 64]
        wr = np.concatenate(list(wr), axis=1)                # [16, ncalls*64]
        wr = np.tile(wr, (8, 1))                             # [128, ncalls*64]
        lc = lrel_p[c * TPC:(c + 1) * TPC].reshape(TPC * K, 128).T
        in_maps.append({
            "rows_w": np.ascontiguousarray(wr),
            "lrel": np.ascontiguousarray(lc.astype(ml_dtypes.bfloat16)),
        })
    return in_maps, K


def kernel(table, event_cell, event_row, _want_trace=False):
    from concourse.bass_utils import run_bass_kernel_spmd

    tabbf = np.ascontiguousarray(
        np.asarray(table, dtype=np.float32).astype(ml_dtypes.bfloat16))
    in_maps, K = _marshal(event_cell, event_row)
    for m in in_maps:
        m["tabbf"] = tabbf

    if K not in _compiled:
        _compiled[K] = _build(K)
    nc = _compiled[K]

    kw = {"trace": True} if _want_trace else {}
    res = run_bass_kernel_spmd(nc, in_maps, core_ids=list(range(NCORES)), **kw)
    full = np.concatenate([res.results[i]["out"] for i in range(NCORES)], axis=0)
    out = full.reshape(B, W, H, L, D).astype(np.float32)
    if _want_trace:
        return out, res
    return out


# revision 8
# speedup vs baseline: 2.7745x; 1.0010x over previous
"""Trainium2 Bass kernel: scatter-add of table rows into a voxel grid.

Computes out[cell] += table[row] for ~1M (cell, row) events, out shape
[B*W*H*L, D] = [131072, 256] fp32.

Strategy (8 NeuronCores, output-sharded):
  - Host: sort events by destination cell; core k owns cells
    [k*16384, (k+1)*16384).  Within a core, cells are grouped into 128
    tiles of 128 cells; each tile's events are padded to K*128 (K =
    global max chunks per tile) so all cores run an identical program.
  - Device (per core): dma_gather the event rows (bf16 table, 1024
    rows per call — SWDGE ring limit), build one-hot [event x cell]
    selection matrices on the vector engine, and accumulate
    one-hot^T @ rows on the PE array into a PSUM tile (fp32
    accumulation), then DMA each [128, 256] cell tile to HBM.
"""

import numpy as np
import ml_dtypes

B, W, H, L, D = 4, 32, 32, 32, 256
NCELLS = B * W * H * L          # 131072
TROWS = 4096
NCORES = 8
CPC = NCELLS // NCORES          # cells per core: 16384
TPC = CPC // 128                # 128-cell tiles per core: 128
GIDX = 1024                     # idxs per dma_gather call (HW ring limit)
GCH = GIDX // 128               # event-chunks per gather call: 8

_compiled = {}


def _build(K):
    import concourse.tile as tile
    from concourse import bacc, mybir

    f32, bf16, i16 = mybir.dt.float32, mybir.dt.bfloat16, mybir.dt.int16
    nch = TPC * K                        # event chunks per core
    ncalls = nch // GCH                  # gather calls per core

    nc = bacc.Bacc("TRN2", target_bir_lowering=False, debug=False,
                   num_devices=NCORES, num_swdge_queues=4)
    tabbf = nc.dram_tensor("tabbf", [TROWS, D], bf16, kind="ExternalInput")
    rows_w = nc.dram_tensor("rows_w", [128, ncalls * (GIDX // 16)], i16,
                            kind="ExternalInput")
    lrel = nc.dram_tensor("lrel", [128, nch], bf16, kind="ExternalInput")
    out = nc.dram_tensor("out", [CPC, D], f32, kind="ExternalOutput")

    with tile.TileContext(nc) as tc:
        with tc.tile_pool(name="const", bufs=1) as constp, \
             tc.tile_pool(name="gbuf", bufs=18) as gpool, \
             tc.tile_pool(name="oh", bufs=12) as ohpool, \
             tc.tile_pool(name="psum", bufs=8, space="PSUM") as pspool, \
             tc.tile_pool(name="stage", bufs=3) as stpool:
            rows_sb = constp.tile([128, ncalls * (GIDX // 16)], i16)
            nc.sync.dma_start(rows_sb[:], rows_w[:])
            lrel_sb = constp.tile([128, nch], bf16)
            nc.sync.dma_start(lrel_sb[:], lrel[:])
            # iota repeated OHB times along free dim for batched one-hot builds
            OHB = 8
            iota_t = constp.tile([128, OHB, 128], bf16)
            nc.gpsimd.iota(iota_t[:], pattern=[[0, OHB], [1, 128]], base=0,
                           channel_multiplier=0,
                           allow_small_or_imprecise_dtypes=True)

            OB = 4      # output tiles batched per DMA
            gt = None
            oh = None
            st = None
            for t in range(TPC):
                ps = pspool.tile([128, D], f32, space="PSUM")
                for j in range(K):
                    c = t * K + j
                    if c % GCH == 0:
                        ci = c // GCH
                        gt = gpool.tile([128, GCH, D], bf16)
                        nc.gpsimd.dma_gather(
                            gt[:], tabbf[:],
                            rows_sb[:, ci * (GIDX // 16):(ci + 1) * (GIDX // 16)],
                            GIDX, GIDX, D, queue_num=ci % 4)
                    if c % OHB == 0:
                        oh = ohpool.tile([128, OHB, 128], bf16)
                        nb = min(OHB, TPC * K - c)
                        nc.vector.tensor_tensor(
                            out=oh[:, :nb, :],
                            in0=lrel_sb[:, c:c + nb, None].to_broadcast(
                                [128, nb, 128]),
                            in1=iota_t[:, :nb, :],
                            op=mybir.AluOpType.is_equal)
                    nc.tensor.matmul(out=ps[:], lhsT=oh[:, c % OHB, :],
                                     rhs=gt[:, c % GCH, :],
                                     start=(j == 0), stop=(j == K - 1))
                if t % OB == 0:
                    st = stpool.tile([128, OB, D], f32)
                nc.any.tensor_copy(st[:, t % OB, :], ps[:])
                if t % OB == OB - 1:
                    t0 = t - (OB - 1)
                    nc.sync.dma_start(
                        out[t0 * 128:(t0 + OB) * 128, :].rearrange(
                            "(q p) d -> p q d", p=128),
                        st[:])
    nc.compile()
    return nc


def _marshal(event_cell, event_row):
    """Sort events by cell, bucket into per-(core,tile) padded streams."""
    ecell = np.asarray(event_cell).astype(np.int64)
    erow = np.asarray(event_row).astype(np.int64)
    order = np.argsort(ecell, kind="stable")
    scell = ecell[order]
    srow = erow[order].astype(np.int16)

    ntiles = NCELLS // 128
    bounds = np.searchsorted(scell, np.arange(ntiles + 1) * 128)
    counts = np.diff(bounds)
    K = max(1, int(-(-int(counts.max()) // 128)))
    # chunks per core must divide evenly into gather calls of GCH chunks
    while (TPC * K) % GCH:
        K += 1
    cap = 128 * K

    rows_p = np.zeros((ntiles, cap), np.int16)
    lrel_p = np.full((ntiles, cap), -1.0, np.float32)
    for t in range(ntiles):
        n = int(counts[t])
        if n:
            s = int(bounds[t])
            rows_p[t, :n] = srow[s:s + n]
            lrel_p[t, :n] = scell[s:s + n] & 127

    in_maps = []
    for c in range(NCORES):
        tok = rows_p[c * TPC:(c + 1) * TPC].reshape(-1)      # TPC*cap events
        # wrapped-by-16 idx layout per 1024-idx gather call, replicated
        # to 128 partitions (8 Q7 cores x 16)
        wr = tok.reshape(-1, GIDX).reshape(-1, 64, 16)       # [ncalls, 64, 16]
        wr = wr.transpose(0, 2, 1).reshape(-1, 16, 64)       # call-major [16 x 64]
        wr = np.concatenate(list(wr), axis=1)                # [16, ncalls*64]
        wr = np.tile(wr, (8, 1))                             # [128, ncalls*64]
        lc = lrel_p[c * TPC:(c + 1) * TPC].reshape(TPC * K, 128).T
        in_maps.append({
            "rows_w": np.ascontiguousarray(wr),
            "lrel": np.ascontiguousarray(lc.astype(ml_dtypes.bfloat16)),
        })
    return in_maps, K


def kernel(table, event_cell, event_row, _want_trace=False):
    from concourse.bass_utils import run_bass_kernel_spmd

    tabbf = np.ascontiguousarray(
        np.asarray(table, dtype=np.float32).astype(ml_dtypes.bfloat16))
    in_maps, K = _marshal(event_cell, event_row)
    for m in in_maps:
        m["tabbf"] = tabbf

    if K not in _compiled:
        _compiled[K] = _build(K)
    nc = _compiled[K]

    kw = {"trace": True} if _want_trace else {}
    res = run_bass_kernel_spmd(nc, in_maps, core_ids=list(range(NCORES)), **kw)
    full = np.concatenate([res.results[i]["out"] for i in range(NCORES)], axis=0)
    out = full.reshape(B, W, H, L, D).astype(np.float32)
    if _want_trace:
        return out, res
    return out


# revision 9
# speedup vs baseline: 3.0879x; 1.1129x over previous
"""Trainium2 Bass kernel: scatter-add of table rows into a voxel grid.

Computes out[cell] += table[row] for ~1M (cell, row) events, out shape
[B*W*H*L, D] = [131072, 256] fp32.

Strategy (8 NeuronCores, output-sharded):
  - Host: sort events by destination cell; core k owns cells
    [k*16384, (k+1)*16384).  Within a core, cells are grouped into 128
    tiles of 128 cells; each tile's events are padded to K*128 (K =
    global max chunks per tile) so all cores run an identical program.
  - Device (per core): dma_gather the event rows (bf16 table, 1024
    rows per call — SWDGE ring limit), build one-hot [event x cell]
    selection matrices on the vector engine, and accumulate
    one-hot^T @ rows on the PE array into a PSUM tile (fp32
    accumulation), then DMA each [128, 256] cell tile to HBM.
"""

import numpy as np
import ml_dtypes

B, W, H, L, D = 4, 32, 32, 32, 256
NCELLS = B * W * H * L          # 131072
TROWS = 4096
NCORES = 8
CPC = NCELLS // NCORES          # cells per core: 16384
TPC = CPC // 128                # 128-cell tiles per core: 128
GIDX = 1024                     # idxs per dma_gather call (HW ring limit)
GCH = GIDX // 128               # event-chunks per gather call: 8

_compiled = {}


def _build(K):
    import concourse.tile as tile
    from concourse import bacc, mybir

    f32, bf16, i16 = mybir.dt.float32, mybir.dt.bfloat16, mybir.dt.int16
    f8 = mybir.dt.float8e4
    nch = TPC * K                        # event chunks per core
    ncalls = nch // GCH                  # gather calls per core

    nc = bacc.Bacc("TRN2", target_bir_lowering=False, debug=False,
                   num_devices=NCORES, num_swdge_queues=4)
    tabbf = nc.dram_tensor("tabbf", [TROWS, D], bf16, kind="ExternalInput")
    rows_w = nc.dram_tensor("rows_w", [128, ncalls * (GIDX // 16)], i16,
                            kind="ExternalInput")
    lrel = nc.dram_tensor("lrel", [128, nch], bf16, kind="ExternalInput")
    out = nc.dram_tensor("out", [CPC, D], f32, kind="ExternalOutput")

    with tile.TileContext(nc) as tc:
        with tc.tile_pool(name="const", bufs=1) as constp, \
             tc.tile_pool(name="gbuf", bufs=18) as gpool, \
             tc.tile_pool(name="oh", bufs=12) as ohpool, \
             tc.tile_pool(name="psum", bufs=8, space="PSUM") as pspool, \
             tc.tile_pool(name="stage", bufs=3) as stpool:
            rows_sb = constp.tile([128, ncalls * (GIDX // 16)], i16)
            seg = ncalls * (GIDX // 16) // 8
            for si in range(8):
                nc.sync.dma_start(rows_sb[:, si * seg:(si + 1) * seg],
                                  rows_w[:, si * seg:(si + 1) * seg])
            lrel_sb = constp.tile([128, nch], bf16)
            nc.sync.dma_start(lrel_sb[:], lrel[:])
            # iota repeated OHB times along free dim for batched one-hot builds
            OHB = 8
            iota_t = constp.tile([128, OHB, 128], bf16)  # bf16 inputs, fp8 out
            nc.gpsimd.iota(iota_t[:], pattern=[[0, OHB], [1, 128]], base=0,
                           channel_multiplier=0,
                           allow_small_or_imprecise_dtypes=True)

            OB = 4      # output tiles batched per DMA
            gt = None
            oh = None
            st = None
            for t in range(TPC):
                ps = pspool.tile([128, D], f32, space="PSUM")
                for j in range(K):
                    c = t * K + j
                    if c % GCH == 0:
                        ci = c // GCH
                        gt = gpool.tile([128, GCH, D], bf16)
                        nc.gpsimd.dma_gather(
                            gt[:], tabbf[:],
                            rows_sb[:, ci * (GIDX // 16):(ci + 1) * (GIDX // 16)],
                            GIDX, GIDX, D, queue_num=ci % 4)
                    if c % OHB == 0:
                        oh = ohpool.tile([128, OHB, 128], f8)
                        nb = min(OHB, TPC * K - c)
                        nc.vector.tensor_tensor(
                            out=oh[:, :nb, :],
                            in0=lrel_sb[:, c:c + nb, None].to_broadcast(
                                [128, nb, 128]),
                            in1=iota_t[:, :nb, :],
                            op=mybir.AluOpType.is_equal)
                    nc.tensor.matmul(out=ps[:], lhsT=oh[:, c % OHB, :],
                                     rhs=gt[:, c % GCH, :],
                                     start=(j == 0), stop=(j == K - 1))
                if t % OB == 0:
                    st = stpool.tile([128, OB, D], f32)
                nc.any.tensor_copy(st[:, t % OB, :], ps[:])
                if t % OB == OB - 1:
                    t0 = t - (OB - 1)
                    nc.sync.dma_start(
                        out[t0 * 128:(t0 + OB) * 128, :].rearrange(
                            "(q p) d -> p q d", p=128),
                        st[:])
    nc.compile()
    return nc


def _marshal(event_cell, event_row):
    """Sort events by cell, bucket into per-(core,tile) padded streams."""
    ecell = np.asarray(event_cell).astype(np.int64)
    erow = np.asarray(event_row).astype(np.int64)
    order = np.argsort(ecell, kind="stable")
    scell = ecell[order]
    srow = erow[order].astype(np.int16)

    ntiles = NCELLS // 128
    bounds = np.searchsorted(scell, np.arange(ntiles + 1) * 128)
    counts = np.diff(bounds)
    K = max(1, int(-(-int(counts.max()) // 128)))
    # chunks per core must divide evenly into gather calls of GCH chunks
    while (TPC * K) % GCH:
        K += 1
    cap = 128 * K

    rows_p = np.zeros((ntiles, cap), np.int16)
    lrel_p = np.full((ntiles, cap), -1.0, np.float32)
    for t in range(ntiles):
        n = int(counts[t])
        if n:
            s = int(bounds[t])
            rows_p[t, :n] = srow[s:s + n]
            lrel_p[t, :n] = scell[s:s + n] & 127

    in_maps = []
    for c in range(NCORES):
        tok = rows_p[c * TPC:(c + 1) * TPC].reshape(-1)      # TPC*cap events
        # wrapped-by-16 idx layout per 1024-idx gather call, replicated
        # to 128 partitions (8 Q7 cores x 16)
        wr = tok.reshape(-1, GIDX).reshape(-1, 64, 16)       # [ncalls, 64, 16]
        wr = wr.transpose(0, 2, 1).reshape(-1, 16, 64)       # call-major [16 x 64]
        wr = np.concatenate(list(wr), axis=1)                # [16, ncalls*64]
        wr = np.tile(wr, (8, 1))                             # [128, ncalls*64]
        lc = lrel_p[c * TPC:(c + 1) * TPC].reshape(TPC * K, 128).T
        in_maps.append({
            "rows_w": np.ascontiguousarray(wr),
            "lrel": np.ascontiguousarray(lc.astype(ml_dtypes.bfloat16)),
        })
    return in_maps, K


def kernel(table, event_cell, event_row, _want_trace=False):
    from concourse.bass_utils import run_bass_kernel_spmd

    tabbf = np.ascontiguousarray(
        np.asarray(table, dtype=np.float32).astype(ml_dtypes.bfloat16))
    in_maps, K = _marshal(event_cell, event_row)
    for m in in_maps:
        m["tabbf"] = tabbf

    if K not in _compiled:
        _compiled[K] = _build(K)
    nc = _compiled[K]

    kw = {"trace": True} if _want_trace else {}
    res = run_bass_kernel_spmd(nc, in_maps, core_ids=list(range(NCORES)), **kw)
    full = np.concatenate([res.results[i]["out"] for i in range(NCORES)], axis=0)
    out = full.reshape(B, W, H, L, D).astype(np.float32)
    if _want_trace:
        return out, res
    return out
